# revision 49
# baseline (speedup 1.0000x reference)
"""Local (7x7 windowed) attention Trainium2 kernel, v2.1.

Problem: B=1, N=4096 (T=4, H=W=32), C=384, 8 heads x hd=48, window 7x7
zero-padded (reference semantics: padded keys score exactly 0 -> weight
exp(0), value 0).

Sharding: t x head-group. Core c owns t-slice c//2 (1024 positions, all
32 rows -- no halo) and heads [4*(c%2), 4*(c%2)+4). Each core computes
its 4 heads' attention + projection partial sum; the host adds the two
partials per t-slice (+ bias).

Device pipeline per core (bf16 matmuls, fp32 PSUM):
  1. q^T/k^T per head-pair (packed 2 heads / 128 partitions), v natural
     [pos, head, hd|1-col] for PV stationary use.
  2. per (head, jt of 4 key rows): S^T = K^T.T @ Q^T (banded query
     spans), exp on ACT, binary window mask multiply on DVE.
  3. per (query-half, pair): O[128, 512] rows 48/112 seeded with n_oob
     via init matmul, accumulates [V|1].T @ E^T; den in rows 48/112.
  4. per half: dens gathered to one PSUM tile (sel matmuls, slots 0/32),
     reciprocal_approx_fast at base partition 0 (custom DVE op breaks at
     nonzero base!), bf16 cast, broadcast matmul, DVE normalize.
  5. proj partial per 128-query tile: P = sum_pr nhat_pr^T.T @ Wp_pr,
     bf16 out DMA. Half 0's phases 4-5 overlap half 1's PV on the PE.
"""

import os

import ml_dtypes
import numpy as np

import concourse.bacc as bacc
import concourse.mybir as mybir
import concourse.tile as tile
from concourse.bass_utils import run_bass_kernel_spmd

F = mybir.dt.float32
R = mybir.dt.float32r
BF = mybir.dt.bfloat16
NPBF = ml_dtypes.bfloat16

NH = 8
HD = 48
T, HH, WW = 4, 32, 32
C = 384
NPOS = T * HH * WW
SCALE = HD ** -0.5

# per j-tile (4 key rows = 128 keys): (q_lo, q_hi, mask_col_offset)
SPANS8 = [
    (0, 224, 96),
    (32, 352, 0),
    (160, 480, 0),
    (288, 608, 0),
    (416, 736, 0),
    (544, 864, 0),
    (672, 992, 0),
    (800, 1024, 0),
]
# PV j-tile slices per query half: (jt, q_lo, q_hi)
PV_HALF = [
    [(0, 0, 224), (1, 32, 352), (2, 160, 480), (3, 288, 512), (4, 416, 512)],
    [(3, 512, 608), (4, 512, 736), (5, 544, 864), (6, 672, 992),
     (7, 800, 1024)],
]
# j-tiles whose mask multiply runs on GpSimd instead of DVE (balance)
GPS_JT = (0, 7)

_CACHE = {}
LAST_RESULT = None


def _build_nc():
    if "nc" in _CACHE:
        return _CACHE["nc"]
    nc = bacc.Bacc("TRN2", target_bir_lowering=False)

    d_xT = [nc.dram_tensor(f"xT{k}", [128, 1024], BF, kind="ExternalInput")
            for k in range(3)]
    # weights blob: wqk [3,4,128] | wv [3,192] | wp [2,384] along free dim
    d_wb = nc.dram_tensor("wblob", [128, 2880], BF, kind="ExternalInput")
    # consts blob: maskbin 320 | sel48 32 | bsel 128
    d_cb = nc.dram_tensor("cblob", [128, 480], BF, kind="ExternalInput")
    # row blob: noob16 1024 | e2 128
    d_rb = nc.dram_tensor("rblob", [1, 1152], BF, kind="ExternalInput")
    d_out = nc.dram_tensor("out", [1024, 384], F, kind="ExternalOutput")

    EXP = mybir.ActivationFunctionType.Exp

    with tile.TileContext(nc) as tc:
        with tc.tile_pool(name="singles", bufs=1) as S:
            wb = S.tile([128, 2880], BF)
            xTs = [S.tile([128, 1024], BF, name=f"xT{k}") for k in range(3)]
            cb = S.tile([128, 480], BF)
            rb = S.tile([1, 1152], BF)

            def wqk_(k, ti):
                o = (k * 4 + ti) * 128
                return wb[:, o:o + 128]

            def wv_(k):
                o = 1536 + k * 192
                return wb[:, o:o + 192]

            def wp_(pr):
                o = 2112 + pr * 384
                return wb[:, o:o + 384]

            def mask_(lo, hi):
                return cb[:, lo:hi]

            sel48_ = cb[:, 320:352]

            def bsel_(pr):
                return cb[32 * pr:32 * pr + 32, 352:480]

            def noob_(hf):
                return rb[:, 512 * hf:512 * hf + 512]

            e2_ = rb[:, 1024:1152]
            qT2 = S.tile([128, 2, 1024], BF)
            kT2 = S.tile([128, 2, 1024], BF)
            vaug = S.tile([128, 8, 4, 64], BF)
            nhat = S.tile([128, 2, 1024], BF)
            recsF = [S.tile([64, 512], F, name=f"recF{i}") for i in range(2)]
            recsB = [S.tile([64, 512], BF, name=f"recB{i}")
                     for i in range(2)]
            oTs = [S.tile([128, 512], BF, name=f"oT{s}") for s in range(4)]

            nc.sync.dma_start(out=wb[:], in_=d_wb[:])
            nc.sync.dma_start(out=xTs[0][:], in_=d_xT[0][:])
            nc.sync.dma_start(out=cb[:], in_=d_cb[:])
            nc.sync.dma_start(out=rb[:], in_=d_rb[:])
            nc.sync.dma_start(out=xTs[1][:], in_=d_xT[1][:])
            nc.sync.dma_start(out=xTs[2][:], in_=d_xT[2][:])

            nc.vector.memset(vaug[:, :, :, 49:64], 0.0)
            nc.vector.memset(vaug[:, :, :, 48:49], 1.0)

            # ---- phase 1: q^T/k^T per head-pair, v natural -----------
            with tc.tile_pool(name="psA", bufs=2, space="PSUM") as psA:
                for ti in range(4):  # 2*pr + s; s: 0=q, 1=k
                    pr, s = divmod(ti, 2)
                    dst = qT2 if s == 0 else kT2
                    for hf in range(2):
                        A = psA.tile([128, 512], F, tag="A")
                        for k in range(3):
                            nc.tensor.matmul(
                                A[:], wqk_(k, ti),
                                xTs[k][:, 512 * hf:512 * hf + 512],
                                start=(k == 0), stop=(k == 2))
                        if (ti + hf) % 2 == 0:
                            nc.scalar.copy(
                                dst[:, pr, 512 * hf:512 * hf + 512], A[:])
                        else:
                            nc.vector.tensor_copy(
                                dst[:, pr, 512 * hf:512 * hf + 512], A[:])
                for pt in range(8):
                    V = psA.tile([128, 192], F, tag="V")
                    for k in range(3):
                        nc.tensor.matmul(
                            V[:], xTs[k][:, 128 * pt:128 * pt + 128],
                            wv_(k), start=(k == 0), stop=(k == 2))
                    if pt % 2 == 0:
                        nc.vector.tensor_copy(
                            vaug[:, pt, :, 0:48],
                            V[:].rearrange("p (h d) -> p h d", h=4))
                    else:
                        nc.scalar.copy(
                            vaug[:, pt, :, 0:48],
                            V[:].rearrange("p (h d) -> p h d", h=4))

            # ---- phase 2: scores, exp, mask for all 4 heads ----------
            with tc.tile_pool(name="psS", bufs=2, space="PSUM") as psS, \
                 tc.tile_pool(name="psO", bufs=1, space="PSUM") as psO, \
                 tc.tile_pool(name="psD", bufs=1, space="PSUM") as psD, \
                 tc.tile_pool(name="psB", bufs=1, space="PSUM") as psB, \
                 tc.tile_pool(name="psP", bufs=2, space="PSUM") as psP, \
                 tc.tile_pool(name="sb2", bufs=4) as sb2, \
                 tc.tile_pool(name="sbo", bufs=2) as sbo:
                eTs = []
                for h in range(4):
                    pr, e = divmod(h, 2)
                    eT = sb2.tile([128, 8, 320], BF, tag="eT", name=f"eT{h}")
                    eTs.append(eT)
                    for jt in range(8):
                        qlo, qhi, mo = SPANS8[jt]
                        spn = qhi - qlo
                        ps = psS.tile([128, 320], F, tag="s")
                        nc.tensor.matmul(
                            ps[:, 0:spn],
                            kT2[64 * e:64 * e + 64, pr,
                                128 * jt:128 * jt + 128],
                            qT2[64 * e:64 * e + 64, pr, qlo:qhi],
                            start=True, stop=True)
                        nc.scalar.activation(eT[:, jt, 0:spn],
                                             ps[:, 0:spn], EXP, scale=SCALE)
                        eng = nc.gpsimd if jt in GPS_JT else nc.vector
                        eng.tensor_mul(eT[:, jt, 0:spn], eT[:, jt, 0:spn],
                                       mask_(mo, mo + spn))

                # ---- phase 3: PV + den gather, both halves -----------
                Ds = []
                for hf in range(2):
                    D = psD.tile([128, 512], F, tag="D", name=f"D{hf}")
                    Ds.append(D)
                    for pr in range(2):
                        s = 2 * pr + hf
                        O = psO.tile([128, 512], F, tag=f"O{pr}",
                                     name=f"O{pr}_{hf}")
                        nc.tensor.matmul(O[:], e2_, noob_(hf),
                                         start=True, stop=False,
                                         skip_group_check=True)
                        pv = PV_HALF[hf]
                        for e in range(2):
                            for i, (jt, lo, hi) in enumerate(pv):
                                last = (e == 1) and (i == len(pv) - 1)
                                elo = lo - SPANS8[jt][0]
                                nc.tensor.matmul(
                                    O[64 * e:64 * e + 64,
                                      lo - 512 * hf:hi - 512 * hf],
                                    vaug[:, jt, 2 * pr + e, :],
                                    eTs[2 * pr + e][:, jt, elo:elo + hi - lo],
                                    start=False, stop=last,
                                    skip_group_check=True)
                        if pr == 0:
                            nc.scalar.copy(oTs[s][:], O[:])
                        else:
                            nc.vector.tensor_copy(oTs[s][:], O[:])
                        nc.tensor.matmul(D[32 * pr:32 * pr + 32, :],
                                         sel48_, oTs[s][:],
                                         start=True, stop=True,
                                         skip_group_check=True)
                    nc.vector.reciprocal_approx_fast(recsF[hf][:],
                                                     D[0:64, :])
                    nc.vector.tensor_copy(recsB[hf][:], recsF[hf][:])

                # ---- phases 4-5: normalize + proj per half -----------
                for hf in range(2):
                    for pr in range(2):
                        s = 2 * pr + hf
                        Bc = psB.tile([128, 512], F, tag="Bc",
                                      name=f"Bc{s}")
                        nc.tensor.matmul(Bc[:], bsel_(pr),
                                         recsB[hf][32 * pr:32 * pr + 32, :],
                                         start=True, stop=True)
                        nc.vector.tensor_mul(
                            nhat[:, pr, 512 * hf:512 * hf + 512],
                            oTs[s][:], Bc[:])
                    for it in range(4 * hf, 4 * hf + 4):
                        P = psP.tile([128, 384], F, tag="P")
                        for pr in range(2):
                            nc.tensor.matmul(
                                P[:], nhat[:, pr, 128 * it:128 * it + 128],
                                wp_(pr), start=(pr == 0), stop=(pr == 1))
                        ot = sbo.tile([128, 384], F, tag="ot")
                        if it % 2 == 0:
                            nc.scalar.copy(ot[:], P[:])
                        else:
                            nc.vector.tensor_copy(ot[:], P[:])
                        nc.sync.dma_start(
                            out=d_out[128 * it:128 * it + 128, :], in_=ot[:])

    nc.compile()
    _CACHE["nc"] = nc
    return nc


def _host_consts():
    if "consts" in _CACHE:
        return _CACHE["consts"]
    mask = np.zeros((128, 320), np.float32)
    for r in range(4):
        for o in range(10):
            if r <= o <= r + 6:
                kx = np.arange(32)[:, None]
                qx = np.arange(32)[None, :]
                mask[32 * r:32 * r + 32, 32 * o:32 * o + 32] = \
                    (np.abs(kx - qx) <= 3).astype(np.float32)
    noob = np.zeros((1, 1024), np.float32)
    for qy in range(32):
        for qx in range(32):
            oy = max(0, 3 - qy) + max(0, qy - 28)
            ox = max(0, 3 - qx) + max(0, qx - 28)
            noob[0, 32 * qy + qx] = 49 - (7 - oy) * (7 - ox)
    e2 = np.zeros((1, 128), np.float32)
    e2[0, 48] = 1.0
    e2[0, 112] = 1.0
    sel48 = np.zeros((128, 32), np.float32)
    sel48[48, 0] = 1.0
    sel48[112, 1] = 1.0
    sel48[48, 2:32] = 1.0  # filler rows stay finite for the reciprocal
    bsel = np.zeros((128, 128), np.float32)
    for pr in range(2):
        bsel[32 * pr, 0:64] = 1.0
        bsel[32 * pr + 1, 64:128] = 1.0
    cblob = np.concatenate([mask, sel48, bsel], axis=1)
    rblob = np.concatenate([noob, e2], axis=1)
    consts = dict(cblob=cblob.astype(NPBF), rblob=rblob.astype(NPBF))
    _CACHE["consts"] = consts
    return consts


def _host_weights(w_qkv, w_proj, hg):
    wqk = np.zeros((128, 3, 4, 128), np.float32)
    for k in range(3):
        rows = slice(k * 128, (k + 1) * 128)
        for pr in range(2):
            for s in range(2):
                g0 = hg * 4 + 2 * pr
                off = 384 * s
                wqk[:, k, 2 * pr + s, 0:48] = \
                    w_qkv[rows, off + 48 * g0:off + 48 * g0 + 48]
                wqk[:, k, 2 * pr + s, 64:112] = \
                    w_qkv[rows, off + 48 * (g0 + 1):off + 48 * (g0 + 1) + 48]
    wv = np.ascontiguousarray(
        w_qkv[:, 768 + 192 * hg:768 + 192 * hg + 192]
        .reshape(3, 128, 192).transpose(1, 0, 2))
    wp = np.zeros((128, 2, 384), np.float32)
    for pr in range(2):
        g0 = hg * 4 + 2 * pr
        wp[0:48, pr, :] = w_proj[48 * g0:48 * g0 + 48, :]
        wp[64:112, pr, :] = w_proj[48 * (g0 + 1):48 * (g0 + 1) + 48, :]
    wblob = np.concatenate([wqk.reshape(128, 1536), wv.reshape(128, 576),
                            wp.reshape(128, 768)], axis=1)
    return dict(wblob=wblob.astype(NPBF))


def kernel(x, w_qkv, w_proj, b_proj, H=32, W=32):
    global LAST_RESULT
    x = np.asarray(x, np.float32)
    w_qkv = np.asarray(w_qkv, np.float32)
    w_proj = np.asarray(w_proj, np.float32)
    b_proj = np.asarray(b_proj, np.float32)
    assert x.shape == (1, NPOS, C) and int(H) == 32 and int(W) == 32

    nc = _build_nc()
    consts = _host_consts()
    wmaps = [_host_weights(w_qkv, w_proj, hg) for hg in range(2)]

    x4 = x[0].reshape(T, 1024, C)
    in_maps = []
    for c in range(8):
        t, hg = c // 2, c % 2
        xT = x4[t].T.reshape(3, 128, 1024).astype(NPBF)
        m = {f"xT{k}": np.ascontiguousarray(xT[k]) for k in range(3)}
        m.update(wmaps[hg])
        m.update(consts)
        in_maps.append(m)

    trace = bool(int(os.environ.get("TRACE", "0")))
    res = run_bass_kernel_spmd(nc, in_maps, core_ids=list(range(8)),
                               trace=trace)
    LAST_RESULT = res
    outs = []
    for t in range(T):
        p0 = res.results[2 * t]["out"].astype(np.float32)
        p1 = res.results[2 * t + 1]["out"].astype(np.float32)
        outs.append(p0 + p1 + b_proj)
    return np.concatenate(outs, axis=0).reshape(1, NPOS, C)


# revision 54
# speedup vs baseline: 1.0387x; 1.0387x over previous
"""Local (7x7 windowed) attention Trainium2 kernel, v2.1.

Problem: B=1, N=4096 (T=4, H=W=32), C=384, 8 heads x hd=48, window 7x7
zero-padded (reference semantics: padded keys score exactly 0 -> weight
exp(0), value 0).

Sharding: t x head-group. Core c owns t-slice c//2 (1024 positions, all
32 rows -- no halo) and heads [4*(c%2), 4*(c%2)+4). Each core computes
its 4 heads' attention + projection partial sum; the host adds the two
partials per t-slice (+ bias).

Device pipeline per core (bf16 matmuls, fp32 PSUM):
  1. q^T/k^T per head-pair (packed 2 heads / 128 partitions), v natural
     [pos, head, hd|1-col] for PV stationary use.
  2. per (head, jt of 4 key rows): S^T = K^T.T @ Q^T (banded query
     spans), exp on ACT, binary window mask multiply on DVE.
  3. per (query-half, pair): O[128, 512] rows 48/112 seeded with n_oob
     via init matmul, accumulates [V|1].T @ E^T; den in rows 48/112.
  4. per half: dens gathered to one PSUM tile (sel matmuls, slots 0/32),
     reciprocal_approx_fast at base partition 0 (custom DVE op breaks at
     nonzero base!), bf16 cast, broadcast matmul, DVE normalize.
  5. proj partial per 128-query tile: P = sum_pr nhat_pr^T.T @ Wp_pr,
     bf16 out DMA. Half 0's phases 4-5 overlap half 1's PV on the PE.
"""

import os

import ml_dtypes
import numpy as np

import concourse.bacc as bacc
import concourse.mybir as mybir
import concourse.tile as tile
from concourse.bass_utils import run_bass_kernel_spmd

F = mybir.dt.float32
R = mybir.dt.float32r
BF = mybir.dt.bfloat16
NPBF = ml_dtypes.bfloat16

NH = 8
HD = 48
T, HH, WW = 4, 32, 32
C = 384
NPOS = T * HH * WW
SCALE = HD ** -0.5

# per j-tile (4 key rows = 128 keys): (q_lo, q_hi, mask_col_offset)
SPANS8 = [
    (0, 224, 96),
    (32, 352, 0),
    (160, 480, 0),
    (288, 608, 0),
    (416, 736, 0),
    (544, 864, 0),
    (672, 992, 0),
    (800, 1024, 0),
]
# PV j-tile slices per query half: (jt, q_lo, q_hi)
PV_HALF = [
    [(0, 0, 224), (1, 32, 352), (2, 160, 480), (3, 288, 512), (4, 416, 512)],
    [(3, 512, 608), (4, 512, 736), (5, 544, 864), (6, 672, 992),
     (7, 800, 1024)],
]
# j-tiles whose mask multiply runs on GpSimd instead of DVE (balance)
GPS_JT = (0, 7)

_CACHE = {}
LAST_RESULT = None


def _build_nc():
    if "nc" in _CACHE:
        return _CACHE["nc"]
    nc = bacc.Bacc("TRN2", target_bir_lowering=False)

    d_xT = [nc.dram_tensor(f"xT{k}", [128, 1024], BF, kind="ExternalInput")
            for k in range(3)]
    # weights: wqk [3,4,128] flat; wvp = wv [3,192] | wp [2,384]
    d_wqkb = nc.dram_tensor("wqkb", [128, 1536], BF, kind="ExternalInput")
    d_wvp = nc.dram_tensor("wvpb", [128, 1344], BF, kind="ExternalInput")
    # consts blob: maskbin 320 | sel48 32 | bsel 128
    d_cb = nc.dram_tensor("cblob", [128, 480], BF, kind="ExternalInput")
    # row blob: noob16 1024 | e2 128
    d_rb = nc.dram_tensor("rblob", [1, 1152], BF, kind="ExternalInput")
    d_out = nc.dram_tensor("out", [1024, 384], BF, kind="ExternalOutput")

    EXP = mybir.ActivationFunctionType.Exp

    with tile.TileContext(nc) as tc:
        with tc.tile_pool(name="singles", bufs=1) as S:
            wqkb = S.tile([128, 1536], BF)
            wvp = S.tile([128, 1344], BF)
            xTs = [S.tile([128, 1024], BF, name=f"xT{k}") for k in range(3)]
            cb = S.tile([128, 480], BF)
            rb = S.tile([1, 1152], BF)

            def wqk_(k, ti):
                o = (k * 4 + ti) * 128
                return wqkb[:, o:o + 128]

            def wv_(k):
                o = k * 192
                return wvp[:, o:o + 192]

            def wp_(pr):
                o = 576 + pr * 384
                return wvp[:, o:o + 384]

            def mask_(lo, hi):
                return cb[:, lo:hi]

            sel48_ = cb[:, 320:352]

            def bsel_(pr):
                return cb[32 * pr:32 * pr + 32, 352:480]

            def noob_(hf):
                return rb[:, 512 * hf:512 * hf + 512]

            e2_ = rb[:, 1024:1152]
            qT2 = S.tile([128, 2, 1024], BF)
            kT2 = S.tile([128, 2, 1024], BF)
            vaug = S.tile([128, 8, 4, 64], BF)
            nhat = S.tile([128, 2, 1024], BF)
            recsF = [S.tile([64, 512], F, name=f"recF{i}") for i in range(2)]
            recsB = [S.tile([64, 512], BF, name=f"recB{i}")
                     for i in range(2)]
            oTs = [S.tile([128, 512], BF, name=f"oT{s}") for s in range(4)]

            nc.scalar.dma_start(out=wqkb[:], in_=d_wqkb[:])
            nc.sync.dma_start(out=xTs[0][:], in_=d_xT[0][:])
            nc.scalar.dma_start(out=rb[:], in_=d_rb[:])
            nc.sync.dma_start(out=xTs[1][:], in_=d_xT[1][:])
            nc.scalar.dma_start(out=wvp[:], in_=d_wvp[:])
            nc.sync.dma_start(out=xTs[2][:], in_=d_xT[2][:])
            nc.sync.dma_start(out=cb[:], in_=d_cb[:])

            nc.vector.memset(vaug[:, :, :, 49:64], 0.0)
            nc.vector.memset(vaug[:, :, :, 48:49], 1.0)

            # ---- phase 1: q^T/k^T per head-pair, v natural -----------
            with tc.tile_pool(name="psA", bufs=2, space="PSUM") as psA:
                for ti in range(4):  # 2*pr + s; s: 0=q, 1=k
                    pr, s = divmod(ti, 2)
                    dst = qT2 if s == 0 else kT2
                    for hf in range(2):
                        A = psA.tile([128, 512], F, tag="A")
                        for k in range(3):
                            nc.tensor.matmul(
                                A[:], wqk_(k, ti),
                                xTs[k][:, 512 * hf:512 * hf + 512],
                                start=(k == 0), stop=(k == 2))
                        if (ti + hf) % 2 == 0:
                            nc.scalar.copy(
                                dst[:, pr, 512 * hf:512 * hf + 512], A[:])
                        else:
                            nc.vector.tensor_copy(
                                dst[:, pr, 512 * hf:512 * hf + 512], A[:])
                for pt in range(8):
                    V = psA.tile([128, 192], F, tag="V")
                    for k in range(3):
                        nc.tensor.matmul(
                            V[:], xTs[k][:, 128 * pt:128 * pt + 128],
                            wv_(k), start=(k == 0), stop=(k == 2))
                    if pt % 2 == 0:
                        nc.vector.tensor_copy(
                            vaug[:, pt, :, 0:48],
                            V[:].rearrange("p (h d) -> p h d", h=4))
                    else:
                        nc.scalar.copy(
                            vaug[:, pt, :, 0:48],
                            V[:].rearrange("p (h d) -> p h d", h=4))

            # ---- phase 2: scores, exp, mask for all 4 heads ----------
            with tc.tile_pool(name="psS", bufs=2, space="PSUM") as psS, \
                 tc.tile_pool(name="psO", bufs=1, space="PSUM") as psO, \
                 tc.tile_pool(name="psD", bufs=1, space="PSUM") as psD, \
                 tc.tile_pool(name="psB", bufs=1, space="PSUM") as psB, \
                 tc.tile_pool(name="psP", bufs=2, space="PSUM") as psP, \
                 tc.tile_pool(name="sb2", bufs=4) as sb2, \
                 tc.tile_pool(name="sbo", bufs=2) as sbo:
                eTs = []
                for h in range(4):
                    pr, e = divmod(h, 2)
                    eT = sb2.tile([128, 8, 320], BF, tag="eT", name=f"eT{h}")
                    eTs.append(eT)
                    for jt in range(8):
                        qlo, qhi, mo = SPANS8[jt]
                        spn = qhi - qlo
                        ps = psS.tile([128, 320], F, tag="s")
                        nc.tensor.matmul(
                            ps[:, 0:spn],
                            kT2[64 * e:64 * e + 64, pr,
                                128 * jt:128 * jt + 128],
                            qT2[64 * e:64 * e + 64, pr, qlo:qhi],
                            start=True, stop=True)
                        nc.scalar.activation(eT[:, jt, 0:spn],
                                             ps[:, 0:spn], EXP, scale=SCALE)
                        eng = nc.gpsimd if jt in GPS_JT else nc.vector
                        eng.tensor_mul(eT[:, jt, 0:spn], eT[:, jt, 0:spn],
                                       mask_(mo, mo + spn))

                # ---- phase 3: PV + den gather, both halves -----------
                Ds = []
                for hf in range(2):
                    D = psD.tile([128, 512], F, tag="D", name=f"D{hf}")
                    Ds.append(D)
                    for pr in range(2):
                        s = 2 * pr + hf
                        O = psO.tile([128, 512], F, tag=f"O{pr}",
                                     name=f"O{pr}_{hf}")
                        nc.tensor.matmul(O[:], e2_, noob_(hf),
                                         start=True, stop=False,
                                         skip_group_check=True)
                        pv = PV_HALF[hf]
                        for e in range(2):
                            for i, (jt, lo, hi) in enumerate(pv):
                                last = (e == 1) and (i == len(pv) - 1)
                                elo = lo - SPANS8[jt][0]
                                nc.tensor.matmul(
                                    O[64 * e:64 * e + 64,
                                      lo - 512 * hf:hi - 512 * hf],
                                    vaug[:, jt, 2 * pr + e, :],
                                    eTs[2 * pr + e][:, jt, elo:elo + hi - lo],
                                    start=False, stop=last,
                                    skip_group_check=True)
                        if pr == 0:
                            nc.scalar.copy(oTs[s][:], O[:])
                        else:
                            nc.vector.tensor_copy(oTs[s][:], O[:])
                        nc.tensor.matmul(D[32 * pr:32 * pr + 32, :],
                                         sel48_, oTs[s][:],
                                         start=True, stop=True,
                                         skip_group_check=True)
                    nc.vector.reciprocal_approx_fast(recsF[hf][:],
                                                     D[0:64, :])
                    nc.vector.tensor_copy(recsB[hf][:], recsF[hf][:])

                # ---- phases 4-5: normalize + proj per half -----------
                for hf in range(2):
                    for pr in range(2):
                        s = 2 * pr + hf
                        Bc = psB.tile([128, 512], F, tag="Bc",
                                      name=f"Bc{s}")
                        nc.tensor.matmul(Bc[:], bsel_(pr),
                                         recsB[hf][32 * pr:32 * pr + 32, :],
                                         start=True, stop=True)
                        nc.vector.tensor_mul(
                            nhat[:, pr, 512 * hf:512 * hf + 512],
                            oTs[s][:], Bc[:])
                    for it in range(4 * hf, 4 * hf + 4):
                        P = psP.tile([128, 384], F, tag="P")
                        for pr in range(2):
                            nc.tensor.matmul(
                                P[:], nhat[:, pr, 128 * it:128 * it + 128],
                                wp_(pr), start=(pr == 0), stop=(pr == 1))
                        ot = sbo.tile([128, 384], BF, tag="ot")
                        if it % 2 == 0:
                            nc.scalar.copy(ot[:], P[:])
                            nc.sync.dma_start(
                                out=d_out[128 * it:128 * it + 128, :],
                                in_=ot[:])
                        else:
                            nc.vector.tensor_copy(ot[:], P[:])
                            nc.scalar.dma_start(
                                out=d_out[128 * it:128 * it + 128, :],
                                in_=ot[:])

    nc.compile()
    _CACHE["nc"] = nc
    return nc


def _host_consts():
    if "consts" in _CACHE:
        return _CACHE["consts"]
    mask = np.zeros((128, 320), np.float32)
    for r in range(4):
        for o in range(10):
            if r <= o <= r + 6:
                kx = np.arange(32)[:, None]
                qx = np.arange(32)[None, :]
                mask[32 * r:32 * r + 32, 32 * o:32 * o + 32] = \
                    (np.abs(kx - qx) <= 3).astype(np.float32)
    noob = np.zeros((1, 1024), np.float32)
    for qy in range(32):
        for qx in range(32):
            oy = max(0, 3 - qy) + max(0, qy - 28)
            ox = max(0, 3 - qx) + max(0, qx - 28)
            noob[0, 32 * qy + qx] = 49 - (7 - oy) * (7 - ox)
    e2 = np.zeros((1, 128), np.float32)
    e2[0, 48] = 1.0
    e2[0, 112] = 1.0
    sel48 = np.zeros((128, 32), np.float32)
    sel48[48, 0] = 1.0
    sel48[112, 1] = 1.0
    sel48[48, 2:32] = 1.0  # filler rows stay finite for the reciprocal
    bsel = np.zeros((128, 128), np.float32)
    for pr in range(2):
        bsel[32 * pr, 0:64] = 1.0
        bsel[32 * pr + 1, 64:128] = 1.0
    cblob = np.concatenate([mask, sel48, bsel], axis=1)
    rblob = np.concatenate([noob, e2], axis=1)
    consts = dict(cblob=cblob.astype(NPBF), rblob=rblob.astype(NPBF))
    _CACHE["consts"] = consts
    return consts


def _host_weights(w_qkv, w_proj, hg):
    wqk = np.zeros((128, 3, 4, 128), np.float32)
    for k in range(3):
        rows = slice(k * 128, (k + 1) * 128)
        for pr in range(2):
            for s in range(2):
                g0 = hg * 4 + 2 * pr
                off = 384 * s
                wqk[:, k, 2 * pr + s, 0:48] = \
                    w_qkv[rows, off + 48 * g0:off + 48 * g0 + 48]
                wqk[:, k, 2 * pr + s, 64:112] = \
                    w_qkv[rows, off + 48 * (g0 + 1):off + 48 * (g0 + 1) + 48]
    wv = np.ascontiguousarray(
        w_qkv[:, 768 + 192 * hg:768 + 192 * hg + 192]
        .reshape(3, 128, 192).transpose(1, 0, 2))
    wp = np.zeros((128, 2, 384), np.float32)
    for pr in range(2):
        g0 = hg * 4 + 2 * pr
        wp[0:48, pr, :] = w_proj[48 * g0:48 * g0 + 48, :]
        wp[64:112, pr, :] = w_proj[48 * (g0 + 1):48 * (g0 + 1) + 48, :]
    wvpb = np.concatenate([wv.reshape(128, 576), wp.reshape(128, 768)],
                          axis=1)
    return dict(wqkb=wqk.reshape(128, 1536).astype(NPBF),
                wvpb=wvpb.astype(NPBF))


def kernel(x, w_qkv, w_proj, b_proj, H=32, W=32):
    global LAST_RESULT
    x = np.asarray(x, np.float32)
    w_qkv = np.asarray(w_qkv, np.float32)
    w_proj = np.asarray(w_proj, np.float32)
    b_proj = np.asarray(b_proj, np.float32)
    assert x.shape == (1, NPOS, C) and int(H) == 32 and int(W) == 32

    nc = _build_nc()
    consts = _host_consts()
    wmaps = [_host_weights(w_qkv, w_proj, hg) for hg in range(2)]

    x4 = x[0].reshape(T, 1024, C)
    in_maps = []
    for c in range(8):
        t, hg = c // 2, c % 2
        xT = x4[t].T.reshape(3, 128, 1024).astype(NPBF)
        m = {f"xT{k}": np.ascontiguousarray(xT[k]) for k in range(3)}
        m.update(wmaps[hg])
        m.update(consts)
        in_maps.append(m)

    trace = bool(int(os.environ.get("TRACE", "0")))
    res = run_bass_kernel_spmd(nc, in_maps, core_ids=list(range(8)),
                               trace=trace)
    LAST_RESULT = res
    outs = []
    for t in range(T):
        p0 = res.results[2 * t]["out"].astype(np.float32)
        p1 = res.results[2 * t + 1]["out"].astype(np.float32)
        outs.append(p0 + p1 + b_proj)
    return np.concatenate(outs, axis=0).reshape(1, NPOS, C)


# revision 56
# speedup vs baseline: 1.0412x; 1.0024x over previous
"""Local (7x7 windowed) attention Trainium2 kernel, v2.1.

Problem: B=1, N=4096 (T=4, H=W=32), C=384, 8 heads x hd=48, window 7x7
zero-padded (reference semantics: padded keys score exactly 0 -> weight
exp(0), value 0).

Sharding: t x head-group. Core c owns t-slice c//2 (1024 positions, all
32 rows -- no halo) and heads [4*(c%2), 4*(c%2)+4). Each core computes
its 4 heads' attention + projection partial sum; the host adds the two
partials per t-slice (+ bias).

Device pipeline per core (bf16 matmuls, fp32 PSUM):
  1. q^T/k^T per head-pair (packed 2 heads / 128 partitions), v natural
     [pos, head, hd|1-col] for PV stationary use.
  2. per (head, jt of 4 key rows): S^T = K^T.T @ Q^T (banded query
     spans), exp on ACT, binary window mask multiply on DVE.
  3. per (query-half, pair): O[128, 512] rows 48/112 seeded with n_oob
     via init matmul, accumulates [V|1].T @ E^T; den in rows 48/112.
  4. per half: dens gathered to one PSUM tile (sel matmuls, slots 0/32),
     reciprocal_approx_fast at base partition 0 (custom DVE op breaks at
     nonzero base!), bf16 cast, broadcast matmul, DVE normalize.
  5. proj partial per 128-query tile: P = sum_pr nhat_pr^T.T @ Wp_pr,
     bf16 out DMA. Half 0's phases 4-5 overlap half 1's PV on the PE.
"""

import os

import ml_dtypes
import numpy as np

import concourse.bacc as bacc
import concourse.mybir as mybir
import concourse.tile as tile
from concourse.bass_utils import run_bass_kernel_spmd

F = mybir.dt.float32
R = mybir.dt.float32r
BF = mybir.dt.bfloat16
NPBF = ml_dtypes.bfloat16

NH = 8
HD = 48
T, HH, WW = 4, 32, 32
C = 384
NPOS = T * HH * WW
SCALE = HD ** -0.5

# per j-tile (4 key rows = 128 keys): (q_lo, q_hi, mask_col_offset)
SPANS8 = [
    (0, 224, 96),
    (32, 352, 0),
    (160, 480, 0),
    (288, 608, 0),
    (416, 736, 0),
    (544, 864, 0),
    (672, 992, 0),
    (800, 1024, 0),
]
# PV j-tile slices per query half: (jt, q_lo, q_hi)
PV_HALF = [
    [(0, 0, 224), (1, 32, 352), (2, 160, 480), (3, 288, 512), (4, 416, 512)],
    [(3, 512, 608), (4, 512, 736), (5, 544, 864), (6, 672, 992),
     (7, 800, 1024)],
]
# j-tiles whose mask multiply runs on GpSimd instead of DVE (balance)
GPS_JT = (0, 7)

_CACHE = {}
LAST_RESULT = None


def _build_nc():
    if "nc" in _CACHE:
        return _CACHE["nc"]
    nc = bacc.Bacc("TRN2", target_bir_lowering=False)

    d_xT = [nc.dram_tensor(f"xT{k}", [128, 1024], BF, kind="ExternalInput")
            for k in range(3)]
    # weights: wqk [3,4,128] flat; wvp = wv [3,192] | wp [2,384]
    d_wqkb = nc.dram_tensor("wqkb", [128, 1536], BF, kind="ExternalInput")
    d_wvp = nc.dram_tensor("wvpb", [128, 1344], BF, kind="ExternalInput")
    # consts blob: maskbin 320 | sel48 32 | bsel 128
    d_cb = nc.dram_tensor("cblob", [128, 480], BF, kind="ExternalInput")
    # row blob: noob16 1024 | e2 128
    d_rb = nc.dram_tensor("rblob", [1, 1152], BF, kind="ExternalInput")
    d_out = nc.dram_tensor("out", [1024, 384], BF, kind="ExternalOutput")

    EXP = mybir.ActivationFunctionType.Exp

    with tile.TileContext(nc) as tc:
        with tc.tile_pool(name="singles", bufs=1) as S:
            wqkb = S.tile([128, 1536], BF)
            wvp = S.tile([128, 1344], BF)
            xTs = [S.tile([128, 1024], BF, name=f"xT{k}") for k in range(3)]
            cb = S.tile([128, 480], BF)
            rb = S.tile([1, 1152], BF)

            def wqk_(k, ti):
                o = (k * 4 + ti) * 128
                return wqkb[:, o:o + 128]

            def wv_(k):
                o = k * 192
                return wvp[:, o:o + 192]

            def wp_(pr):
                o = 576 + pr * 384
                return wvp[:, o:o + 384]

            def mask_(lo, hi):
                return cb[:, lo:hi]

            sel48_ = cb[:, 320:352]

            def bsel_(pr):
                return cb[32 * pr:32 * pr + 32, 352:480]

            def noob_(hf):
                return rb[:, 512 * hf:512 * hf + 512]

            e2_ = rb[:, 1024:1152]
            qT2 = S.tile([128, 2, 1024], BF)
            kT2 = S.tile([128, 2, 1024], BF)
            vaug = S.tile([128, 8, 4, 64], BF)
            nhat = S.tile([128, 2, 1024], BF)
            recsF = [S.tile([64, 512], F, name=f"recF{i}") for i in range(2)]
            recsB = [S.tile([64, 512], BF, name=f"recB{i}")
                     for i in range(2)]
            oTs = [S.tile([128, 512], BF, name=f"oT{s}") for s in range(4)]

            nc.scalar.dma_start(out=wqkb[:], in_=d_wqkb[:])
            nc.sync.dma_start(out=xTs[0][:], in_=d_xT[0][:])
            nc.scalar.dma_start(out=rb[:], in_=d_rb[:])
            nc.sync.dma_start(out=xTs[1][:], in_=d_xT[1][:])
            nc.scalar.dma_start(out=wvp[:], in_=d_wvp[:])
            nc.sync.dma_start(out=xTs[2][:], in_=d_xT[2][:])
            nc.sync.dma_start(out=cb[:], in_=d_cb[:])

            nc.vector.memset(vaug[:, :, :, 49:64], 0.0)
            nc.vector.memset(vaug[:, :, :, 48:49], 1.0)

            # ---- phase 1: q^T/k^T per head-pair, v natural -----------
            with tc.tile_pool(name="psA", bufs=2, space="PSUM") as psA:
                for ti in range(4):  # 2*pr + s; s: 0=q, 1=k
                    pr, s = divmod(ti, 2)
                    dst = qT2 if s == 0 else kT2
                    for hf in range(2):
                        A = psA.tile([128, 512], F, tag="A")
                        for k in range(3):
                            nc.tensor.matmul(
                                A[:], wqk_(k, ti),
                                xTs[k][:, 512 * hf:512 * hf + 512],
                                start=(k == 0), stop=(k == 2))
                        if (ti + hf) % 2 == 0:
                            nc.scalar.copy(
                                dst[:, pr, 512 * hf:512 * hf + 512], A[:])
                        else:
                            nc.vector.tensor_copy(
                                dst[:, pr, 512 * hf:512 * hf + 512], A[:])
                for pt in range(8):
                    V = psA.tile([128, 192], F, tag="V")
                    for k in range(3):
                        nc.tensor.matmul(
                            V[:], xTs[k][:, 128 * pt:128 * pt + 128],
                            wv_(k), start=(k == 0), stop=(k == 2))
                    if pt % 2 == 0:
                        nc.vector.tensor_copy(
                            vaug[:, pt, :, 0:48],
                            V[:].rearrange("p (h d) -> p h d", h=4))
                    else:
                        nc.scalar.copy(
                            vaug[:, pt, :, 0:48],
                            V[:].rearrange("p (h d) -> p h d", h=4))

            # ---- phase 2: scores, exp, mask for all 4 heads ----------
            with tc.tile_pool(name="psS", bufs=2, space="PSUM") as psS, \
                 tc.tile_pool(name="psO", bufs=1, space="PSUM") as psO, \
                 tc.tile_pool(name="psD", bufs=1, space="PSUM") as psD, \
                 tc.tile_pool(name="psB", bufs=1, space="PSUM") as psB, \
                 tc.tile_pool(name="psP", bufs=2, space="PSUM") as psP, \
                 tc.tile_pool(name="sb2", bufs=4) as sb2, \
                 tc.tile_pool(name="sbo", bufs=2) as sbo:
                eTs = []
                for h in range(4):
                    pr, e = divmod(h, 2)
                    eT = sb2.tile([128, 8, 320], BF, tag="eT", name=f"eT{h}")
                    eTs.append(eT)
                    for jt in range(8):
                        qlo, qhi, mo = SPANS8[jt]
                        spn = qhi - qlo
                        ps = psS.tile([128, 320], F, tag="s")
                        nc.tensor.matmul(
                            ps[:, 0:spn],
                            kT2[64 * e:64 * e + 64, pr,
                                128 * jt:128 * jt + 128],
                            qT2[64 * e:64 * e + 64, pr, qlo:qhi],
                            start=True, stop=True)
                        nc.scalar.activation(eT[:, jt, 0:spn],
                                             ps[:, 0:spn], EXP, scale=SCALE)
                        eng = nc.gpsimd if jt in GPS_JT else nc.vector
                        eng.tensor_mul(eT[:, jt, 0:spn], eT[:, jt, 0:spn],
                                       mask_(mo, mo + spn))

                # ---- phase 3: PV + den gather, both halves -----------
                Ds = []
                for hf in range(2):
                    D = psD.tile([128, 512], F, tag="D", name=f"D{hf}")
                    Ds.append(D)
                    for pr in range(2):
                        s = 2 * pr + hf
                        O = psO.tile([128, 512], F, tag=f"O{pr}",
                                     name=f"O{pr}_{hf}")
                        nc.tensor.matmul(O[:], e2_, noob_(hf),
                                         start=True, stop=False,
                                         skip_group_check=True)
                        pv = PV_HALF[hf]
                        for e in range(2):
                            for i, (jt, lo, hi) in enumerate(pv):
                                last = (e == 1) and (i == len(pv) - 1)
                                elo = lo - SPANS8[jt][0]
                                nc.tensor.matmul(
                                    O[64 * e:64 * e + 64,
                                      lo - 512 * hf:hi - 512 * hf],
                                    vaug[:, jt, 2 * pr + e, :],
                                    eTs[2 * pr + e][:, jt, elo:elo + hi - lo],
                                    start=False, stop=last,
                                    skip_group_check=True)
                        nc.vector.tensor_copy(oTs[s][:], O[:])
                        nc.tensor.matmul(D[32 * pr:32 * pr + 32, :],
                                         sel48_, oTs[s][:],
                                         start=True, stop=True,
                                         skip_group_check=True)
                    nc.vector.reciprocal_approx_fast(recsF[hf][:],
                                                     D[0:64, :])
                    nc.vector.tensor_copy(recsB[hf][:], recsF[hf][:])

                # ---- phases 4-5: normalize + proj per half -----------
                for hf in range(2):
                    for pr in range(2):
                        s = 2 * pr + hf
                        Bc = psB.tile([128, 512], F, tag="Bc",
                                      name=f"Bc{s}")
                        nc.tensor.matmul(Bc[:], bsel_(pr),
                                         recsB[hf][32 * pr:32 * pr + 32, :],
                                         start=True, stop=True)
                        nc.vector.tensor_mul(
                            nhat[:, pr, 512 * hf:512 * hf + 512],
                            oTs[s][:], Bc[:])
                    for it in range(4 * hf, 4 * hf + 4):
                        P = psP.tile([128, 384], F, tag="P")
                        for pr in range(2):
                            nc.tensor.matmul(
                                P[:], nhat[:, pr, 128 * it:128 * it + 128],
                                wp_(pr), start=(pr == 0), stop=(pr == 1))
                        ot = sbo.tile([128, 384], BF, tag="ot")
                        if it % 2 == 0:
                            nc.scalar.copy(ot[:], P[:])
                        else:
                            nc.vector.tensor_copy(ot[:], P[:])
                        nc.sync.dma_start(
                            out=d_out[128 * it:128 * it + 128, :], in_=ot[:])

    nc.compile()
    _CACHE["nc"] = nc
    return nc


def _host_consts():
    if "consts" in _CACHE:
        return _CACHE["consts"]
    mask = np.zeros((128, 320), np.float32)
    for r in range(4):
        for o in range(10):
            if r <= o <= r + 6:
                kx = np.arange(32)[:, None]
                qx = np.arange(32)[None, :]
                mask[32 * r:32 * r + 32, 32 * o:32 * o + 32] = \
                    (np.abs(kx - qx) <= 3).astype(np.float32)
    noob = np.zeros((1, 1024), np.float32)
    for qy in range(32):
        for qx in range(32):
            oy = max(0, 3 - qy) + max(0, qy - 28)
            ox = max(0, 3 - qx) + max(0, qx - 28)
            noob[0, 32 * qy + qx] = 49 - (7 - oy) * (7 - ox)
    e2 = np.zeros((1, 128), np.float32)
    e2[0, 48] = 1.0
    e2[0, 112] = 1.0
    sel48 = np.zeros((128, 32), np.float32)
    sel48[48, 0] = 1.0
    sel48[112, 1] = 1.0
    sel48[48, 2:32] = 1.0  # filler rows stay finite for the reciprocal
    bsel = np.zeros((128, 128), np.float32)
    for pr in range(2):
        bsel[32 * pr, 0:64] = 1.0
        bsel[32 * pr + 1, 64:128] = 1.0
    cblob = np.concatenate([mask, sel48, bsel], axis=1)
    rblob = np.concatenate([noob, e2], axis=1)
    consts = dict(cblob=cblob.astype(NPBF), rblob=rblob.astype(NPBF))
    _CACHE["consts"] = consts
    return consts


def _host_weights(w_qkv, w_proj, hg):
    wqk = np.zeros((128, 3, 4, 128), np.float32)
    for k in range(3):
        rows = slice(k * 128, (k + 1) * 128)
        for pr in range(2):
            for s in range(2):
                g0 = hg * 4 + 2 * pr
                off = 384 * s
                wqk[:, k, 2 * pr + s, 0:48] = \
                    w_qkv[rows, off + 48 * g0:off + 48 * g0 + 48]
                wqk[:, k, 2 * pr + s, 64:112] = \
                    w_qkv[rows, off + 48 * (g0 + 1):off + 48 * (g0 + 1) + 48]
    wv = np.ascontiguousarray(
        w_qkv[:, 768 + 192 * hg:768 + 192 * hg + 192]
        .reshape(3, 128, 192).transpose(1, 0, 2))
    wp = np.zeros((128, 2, 384), np.float32)
    for pr in range(2):
        g0 = hg * 4 + 2 * pr
        wp[0:48, pr, :] = w_proj[48 * g0:48 * g0 + 48, :]
        wp[64:112, pr, :] = w_proj[48 * (g0 + 1):48 * (g0 + 1) + 48, :]
    wvpb = np.concatenate([wv.reshape(128, 576), wp.reshape(128, 768)],
                          axis=1)
    return dict(wqkb=wqk.reshape(128, 1536).astype(NPBF),
                wvpb=wvpb.astype(NPBF))


def kernel(x, w_qkv, w_proj, b_proj, H=32, W=32):
    global LAST_RESULT
    x = np.asarray(x, np.float32)
    w_qkv = np.asarray(w_qkv, np.float32)
    w_proj = np.asarray(w_proj, np.float32)
    b_proj = np.asarray(b_proj, np.float32)
    assert x.shape == (1, NPOS, C) and int(H) == 32 and int(W) == 32

    nc = _build_nc()
    consts = _host_consts()
    wmaps = [_host_weights(w_qkv, w_proj, hg) for hg in range(2)]

    x4 = x[0].reshape(T, 1024, C)
    in_maps = []
    for c in range(8):
        t, hg = c // 2, c % 2
        xT = x4[t].T.reshape(3, 128, 1024).astype(NPBF)
        m = {f"xT{k}": np.ascontiguousarray(xT[k]) for k in range(3)}
        m.update(wmaps[hg])
        m.update(consts)
        in_maps.append(m)

    trace = bool(int(os.environ.get("TRACE", "0")))
    res = run_bass_kernel_spmd(nc, in_maps, core_ids=list(range(8)),
                               trace=trace)
    LAST_RESULT = res
    outs = []
    for t in range(T):
        p0 = res.results[2 * t]["out"].astype(np.float32)
        p1 = res.results[2 * t + 1]["out"].astype(np.float32)
        outs.append(p0 + p1 + b_proj)
    return np.concatenate(outs, axis=0).reshape(1, NPOS, C)


# revision 57
# speedup vs baseline: 1.0509x; 1.0093x over previous
"""Local (7x7 windowed) attention Trainium2 kernel, v2.1.

Problem: B=1, N=4096 (T=4, H=W=32), C=384, 8 heads x hd=48, window 7x7
zero-padded (reference semantics: padded keys score exactly 0 -> weight
exp(0), value 0).

Sharding: t x head-group. Core c owns t-slice c//2 (1024 positions, all
32 rows -- no halo) and heads [4*(c%2), 4*(c%2)+4). Each core computes
its 4 heads' attention + projection partial sum; the host adds the two
partials per t-slice (+ bias).

Device pipeline per core (bf16 matmuls, fp32 PSUM):
  1. q^T/k^T per head-pair (packed 2 heads / 128 partitions), v natural
     [pos, head, hd|1-col] for PV stationary use.
  2. per (head, jt of 4 key rows): S^T = K^T.T @ Q^T (banded query
     spans), exp on ACT, binary window mask multiply on DVE.
  3. per (query-half, pair): O[128, 512] rows 48/112 seeded with n_oob
     via init matmul, accumulates [V|1].T @ E^T; den in rows 48/112.
  4. per half: dens gathered to one PSUM tile (sel matmuls, slots 0/32),
     reciprocal_approx_fast at base partition 0 (custom DVE op breaks at
     nonzero base!), bf16 cast, broadcast matmul, DVE normalize.
  5. proj partial per 128-query tile: P = sum_pr nhat_pr^T.T @ Wp_pr,
     bf16 out DMA. Half 0's phases 4-5 overlap half 1's PV on the PE.
"""

import os

import ml_dtypes
import numpy as np

import concourse.bacc as bacc
import concourse.mybir as mybir
import concourse.tile as tile
from concourse.bass_utils import run_bass_kernel_spmd

F = mybir.dt.float32
R = mybir.dt.float32r
BF = mybir.dt.bfloat16
NPBF = ml_dtypes.bfloat16

NH = 8
HD = 48
T, HH, WW = 4, 32, 32
C = 384
NPOS = T * HH * WW
SCALE = HD ** -0.5

# per j-tile (4 key rows = 128 keys): (q_lo, q_hi, mask_col_offset)
SPANS8 = [
    (0, 224, 96),
    (32, 352, 0),
    (160, 480, 0),
    (288, 608, 0),
    (416, 736, 0),
    (544, 864, 0),
    (672, 992, 0),
    (800, 1024, 0),
]
# PV j-tile slices per query half: (jt, q_lo, q_hi)
PV_HALF = [
    [(0, 0, 224), (1, 32, 352), (2, 160, 480), (3, 288, 512), (4, 416, 512)],
    [(3, 512, 608), (4, 512, 736), (5, 544, 864), (6, 672, 992),
     (7, 800, 1024)],
]
# j-tiles whose mask multiply runs on GpSimd instead of DVE (balance)
GPS_JT = (0, 7)

_CACHE = {}
LAST_RESULT = None


def _build_nc():
    if "nc" in _CACHE:
        return _CACHE["nc"]
    nc = bacc.Bacc("TRN2", target_bir_lowering=False)

    d_xT = [nc.dram_tensor(f"xT{k}", [128, 1024], BF, kind="ExternalInput")
            for k in range(3)]
    # weights: wqk [3,4,128] flat; wvp = wv [3,192] | wp [2,384]
    d_wqkb = nc.dram_tensor("wqkb", [128, 1536], BF, kind="ExternalInput")
    d_wvp = nc.dram_tensor("wvpb", [128, 1344], BF, kind="ExternalInput")
    # consts blob: maskbin 320 | sel48 32 | bsel 128
    d_cb = nc.dram_tensor("cblob", [128, 480], BF, kind="ExternalInput")
    # row blob: noob16 1024 | e2 128
    d_rb = nc.dram_tensor("rblob", [1, 1152], BF, kind="ExternalInput")
    d_out = nc.dram_tensor("out", [1024, 384], BF, kind="ExternalOutput")

    EXP = mybir.ActivationFunctionType.Exp

    with tile.TileContext(nc) as tc:
        with tc.tile_pool(name="singles", bufs=1) as S:
            wqkb = S.tile([128, 1536], BF)
            wvp = S.tile([128, 1344], BF)
            xTs = [S.tile([128, 1024], BF, name=f"xT{k}") for k in range(3)]
            cb = S.tile([128, 480], BF)
            rb = S.tile([1, 1152], BF)

            def wqk_(k, ti):
                o = (k * 4 + ti) * 128
                return wqkb[:, o:o + 128]

            def wv_(k):
                o = k * 192
                return wvp[:, o:o + 192]

            def wp_(pr):
                o = 576 + pr * 384
                return wvp[:, o:o + 384]

            def mask_(lo, hi):
                return cb[:, lo:hi]

            sel48_ = cb[:, 320:352]

            def bsel_(pr):
                return cb[32 * pr:32 * pr + 32, 352:480]

            def noob_(hf):
                return rb[:, 512 * hf:512 * hf + 512]

            e2_ = rb[:, 1024:1152]
            qT2 = S.tile([128, 2, 1024], BF)
            kT2 = S.tile([128, 2, 1024], BF)
            vaug = S.tile([128, 8, 4, 64], BF)
            nhat = S.tile([128, 2, 1024], BF)
            recsF = [S.tile([64, 512], F, name=f"recF{i}") for i in range(2)]
            recsB = [S.tile([64, 512], BF, name=f"recB{i}")
                     for i in range(2)]
            oTs = [S.tile([128, 512], BF, name=f"oT{s}") for s in range(4)]

            nc.sync.dma_start(out=wqkb[:], in_=d_wqkb[:])
            nc.sync.dma_start(out=cb[:], in_=d_cb[:])
            nc.scalar.dma_start(out=xTs[0][:], in_=d_xT[0][:])
            nc.scalar.dma_start(out=rb[:], in_=d_rb[:])
            nc.scalar.dma_start(out=xTs[1][:], in_=d_xT[1][:])
            nc.scalar.dma_start(out=xTs[2][:], in_=d_xT[2][:])
            nc.scalar.dma_start(out=wvp[:], in_=d_wvp[:])

            nc.vector.memset(vaug[:, :, :, 49:64], 0.0)
            nc.vector.memset(vaug[:, :, :, 48:49], 1.0)

            # ---- phase 1: q^T/k^T per head-pair, v natural -----------
            with tc.tile_pool(name="psA", bufs=2, space="PSUM") as psA:
                for ti in range(4):  # 2*pr + s; s: 0=q, 1=k
                    pr, s = divmod(ti, 2)
                    dst = qT2 if s == 0 else kT2
                    for hf in range(2):
                        A = psA.tile([128, 512], F, tag="A")
                        for k in range(3):
                            nc.tensor.matmul(
                                A[:], wqk_(k, ti),
                                xTs[k][:, 512 * hf:512 * hf + 512],
                                start=(k == 0), stop=(k == 2))
                        if (ti + hf) % 2 == 0:
                            nc.scalar.copy(
                                dst[:, pr, 512 * hf:512 * hf + 512], A[:])
                        else:
                            nc.vector.tensor_copy(
                                dst[:, pr, 512 * hf:512 * hf + 512], A[:])
                for pt in range(8):
                    V = psA.tile([128, 192], F, tag="V")
                    for k in range(3):
                        nc.tensor.matmul(
                            V[:], xTs[k][:, 128 * pt:128 * pt + 128],
                            wv_(k), start=(k == 0), stop=(k == 2))
                    if pt % 2 == 0:
                        nc.vector.tensor_copy(
                            vaug[:, pt, :, 0:48],
                            V[:].rearrange("p (h d) -> p h d", h=4))
                    else:
                        nc.scalar.copy(
                            vaug[:, pt, :, 0:48],
                            V[:].rearrange("p (h d) -> p h d", h=4))

            # ---- phase 2: scores, exp, mask for all 4 heads ----------
            with tc.tile_pool(name="psS", bufs=2, space="PSUM") as psS, \
                 tc.tile_pool(name="psO", bufs=1, space="PSUM") as psO, \
                 tc.tile_pool(name="psD", bufs=1, space="PSUM") as psD, \
                 tc.tile_pool(name="psB", bufs=1, space="PSUM") as psB, \
                 tc.tile_pool(name="psP", bufs=2, space="PSUM") as psP, \
                 tc.tile_pool(name="sb2", bufs=4) as sb2, \
                 tc.tile_pool(name="sbo", bufs=2) as sbo:
                eTs = []
                for h in range(4):
                    pr, e = divmod(h, 2)
                    eT = sb2.tile([128, 8, 320], BF, tag="eT", name=f"eT{h}")
                    eTs.append(eT)
                    for jt in range(8):
                        qlo, qhi, mo = SPANS8[jt]
                        spn = qhi - qlo
                        ps = psS.tile([128, 320], F, tag="s")
                        nc.tensor.matmul(
                            ps[:, 0:spn],
                            kT2[64 * e:64 * e + 64, pr,
                                128 * jt:128 * jt + 128],
                            qT2[64 * e:64 * e + 64, pr, qlo:qhi],
                            start=True, stop=True)
                        nc.scalar.activation(eT[:, jt, 0:spn],
                                             ps[:, 0:spn], EXP, scale=SCALE)
                        eng = nc.gpsimd if jt in GPS_JT else nc.vector
                        eng.tensor_mul(eT[:, jt, 0:spn], eT[:, jt, 0:spn],
                                       mask_(mo, mo + spn))

                # ---- phase 3: PV + den gather, both halves -----------
                Ds = []
                for hf in range(2):
                    D = psD.tile([128, 512], F, tag="D", name=f"D{hf}")
                    Ds.append(D)
                    for pr in range(2):
                        s = 2 * pr + hf
                        O = psO.tile([128, 512], F, tag=f"O{pr}",
                                     name=f"O{pr}_{hf}")
                        nc.tensor.matmul(O[:], e2_, noob_(hf),
                                         start=True, stop=False,
                                         skip_group_check=True)
                        pv = PV_HALF[hf]
                        for e in range(2):
                            for i, (jt, lo, hi) in enumerate(pv):
                                last = (e == 1) and (i == len(pv) - 1)
                                elo = lo - SPANS8[jt][0]
                                nc.tensor.matmul(
                                    O[64 * e:64 * e + 64,
                                      lo - 512 * hf:hi - 512 * hf],
                                    vaug[:, jt, 2 * pr + e, :],
                                    eTs[2 * pr + e][:, jt, elo:elo + hi - lo],
                                    start=False, stop=last,
                                    skip_group_check=True)
                        nc.vector.tensor_copy(oTs[s][:], O[:])
                        nc.tensor.matmul(D[32 * pr:32 * pr + 32, :],
                                         sel48_, oTs[s][:],
                                         start=True, stop=True,
                                         skip_group_check=True)
                    nc.vector.reciprocal_approx_fast(recsF[hf][:],
                                                     D[0:64, :])
                    nc.vector.tensor_copy(recsB[hf][:], recsF[hf][:])

                # ---- phases 4-5: normalize + proj per half -----------
                for hf in range(2):
                    for pr in range(2):
                        s = 2 * pr + hf
                        Bc = psB.tile([128, 512], F, tag="Bc",
                                      name=f"Bc{s}")
                        nc.tensor.matmul(Bc[:], bsel_(pr),
                                         recsB[hf][32 * pr:32 * pr + 32, :],
                                         start=True, stop=True)
                        nc.vector.tensor_mul(
                            nhat[:, pr, 512 * hf:512 * hf + 512],
                            oTs[s][:], Bc[:])
                    for it in range(4 * hf, 4 * hf + 4):
                        P = psP.tile([128, 384], F, tag="P")
                        for pr in range(2):
                            nc.tensor.matmul(
                                P[:], nhat[:, pr, 128 * it:128 * it + 128],
                                wp_(pr), start=(pr == 0), stop=(pr == 1))
                        ot = sbo.tile([128, 384], BF, tag="ot")
                        if it % 2 == 0:
                            nc.scalar.copy(ot[:], P[:])
                        else:
                            nc.vector.tensor_copy(ot[:], P[:])
                        nc.sync.dma_start(
                            out=d_out[128 * it:128 * it + 128, :], in_=ot[:])

    nc.compile()
    _CACHE["nc"] = nc
    return nc


def _host_consts():
    if "consts" in _CACHE:
        return _CACHE["consts"]
    mask = np.zeros((128, 320), np.float32)
    for r in range(4):
        for o in range(10):
            if r <= o <= r + 6:
                kx = np.arange(32)[:, None]
                qx = np.arange(32)[None, :]
                mask[32 * r:32 * r + 32, 32 * o:32 * o + 32] = \
                    (np.abs(kx - qx) <= 3).astype(np.float32)
    noob = np.zeros((1, 1024), np.float32)
    for qy in range(32):
        for qx in range(32):
            oy = max(0, 3 - qy) + max(0, qy - 28)
            ox = max(0, 3 - qx) + max(0, qx - 28)
            noob[0, 32 * qy + qx] = 49 - (7 - oy) * (7 - ox)
    e2 = np.zeros((1, 128), np.float32)
    e2[0, 48] = 1.0
    e2[0, 112] = 1.0
    sel48 = np.zeros((128, 32), np.float32)
    sel48[48, 0] = 1.0
    sel48[112, 1] = 1.0
    sel48[48, 2:32] = 1.0  # filler rows stay finite for the reciprocal
    bsel = np.zeros((128, 128), np.float32)
    for pr in range(2):
        bsel[32 * pr, 0:64] = 1.0
        bsel[32 * pr + 1, 64:128] = 1.0
    cblob = np.concatenate([mask, sel48, bsel], axis=1)
    rblob = np.concatenate([noob, e2], axis=1)
    consts = dict(cblob=cblob.astype(NPBF), rblob=rblob.astype(NPBF))
    _CACHE["consts"] = consts
    return consts


def _host_weights(w_qkv, w_proj, hg):
    wqk = np.zeros((128, 3, 4, 128), np.float32)
    for k in range(3):
        rows = slice(k * 128, (k + 1) * 128)
        for pr in range(2):
            for s in range(2):
                g0 = hg * 4 + 2 * pr
                off = 384 * s
                wqk[:, k, 2 * pr + s, 0:48] = \
                    w_qkv[rows, off + 48 * g0:off + 48 * g0 + 48]
                wqk[:, k, 2 * pr + s, 64:112] = \
                    w_qkv[rows, off + 48 * (g0 + 1):off + 48 * (g0 + 1) + 48]
    wv = np.ascontiguousarray(
        w_qkv[:, 768 + 192 * hg:768 + 192 * hg + 192]
        .reshape(3, 128, 192).transpose(1, 0, 2))
    wp = np.zeros((128, 2, 384), np.float32)
    for pr in range(2):
        g0 = hg * 4 + 2 * pr
        wp[0:48, pr, :] = w_proj[48 * g0:48 * g0 + 48, :]
        wp[64:112, pr, :] = w_proj[48 * (g0 + 1):48 * (g0 + 1) + 48, :]
    wvpb = np.concatenate([wv.reshape(128, 576), wp.reshape(128, 768)],
                          axis=1)
    return dict(wqkb=wqk.reshape(128, 1536).astype(NPBF),
                wvpb=wvpb.astype(NPBF))


def kernel(x, w_qkv, w_proj, b_proj, H=32, W=32):
    global LAST_RESULT
    x = np.asarray(x, np.float32)
    w_qkv = np.asarray(w_qkv, np.float32)
    w_proj = np.asarray(w_proj, np.float32)
    b_proj = np.asarray(b_proj, np.float32)
    assert x.shape == (1, NPOS, C) and int(H) == 32 and int(W) == 32

    nc = _build_nc()
    consts = _host_consts()
    wmaps = [_host_weights(w_qkv, w_proj, hg) for hg in range(2)]

    x4 = x[0].reshape(T, 1024, C)
    in_maps = []
    for c in range(8):
        t, hg = c // 2, c % 2
        xT = x4[t].T.reshape(3, 128, 1024).astype(NPBF)
        m = {f"xT{k}": np.ascontiguousarray(xT[k]) for k in range(3)}
        m.update(wmaps[hg])
        m.update(consts)
        in_maps.append(m)

    trace = bool(int(os.environ.get("TRACE", "0")))
    res = run_bass_kernel_spmd(nc, in_maps, core_ids=list(range(8)),
                               trace=trace)
    LAST_RESULT = res
    outs = []
    for t in range(T):
        p0 = res.results[2 * t]["out"].astype(np.float32)
        p1 = res.results[2 * t + 1]["out"].astype(np.float32)
        outs.append(p0 + p1 + b_proj)
    return np.concatenate(outs, axis=0).reshape(1, NPOS, C)
